# revision 59
# baseline (speedup 1.0000x reference)
"""Trainium2 Bass kernel for nn_IntraClassLoss (segment_reduce).

Math: inputs [B,C,H,W] logits, targets [B,H,W] int labels, C=4.
probs = softmax(inputs, axis=1); for classes c in 1..C-1:
  cnt_c = #pixels with target==c
  S1_c  = sum over those pixels of p_c
  S2_c  = sum over those pixels of p_c^2
  mean_c = S1_c/(cnt_c+eps); var_c = (S2_c - 2*mean_c*S1_c + cnt_c*mean_c^2)/(cnt_c+eps)
  loss = sum_{c: cnt_c>0} var_c / (C-1)

Sharding: data-parallel over batch, 2 batches per core on 8 cores. Each core
reduces its shard to per-class S1/S2 partials ([128, n_chunks] columns), which
are DMA'd out and finished on the host (no collectives needed).

The per-chunk work (chunk = [128, N] slice of one batch's planes) is spread
over all engines so each stays under the ~7.28us/chunk DMA floor, using a
3-slot software pipeline: slot j issues stage A of chunk j, stage B of chunk
j-1, stage C2 of chunk j-2, and stage C1 of chunk j-1 (class 1's mask only
needs the first Pool mul, so it runs a slot earlier, shrinking the drain):
  A  | ACT : e_c = exp(x_c) -> bf16                              (4x 1038ns)
  A  | PE  : den = sum_c e_c via identity-matmul accum (PSUM)    (8x ~390ns)
  B  | DVE : r32 = 1/den (reciprocal_approx_fast)                (1192ns)
  B  | Pool: p_c = e_c * r32 (mixed bf16*f32 -> bf16), c=1..3    (3x 2127)
  C1 | DVE : a_1 = (t==1)*p_1, fused accum_out -> S1             (1127)
  C1 | ACT : Square(a_1) with accum_out -> S2                    (1225)
  C2 | DVE : a_c = (t==c)*p_c, accum_out -> S1, c=2,3            (2x 1127)
  C2 | ACT : Square(a_2) with accum_out -> S2                    (1225)
  C2 | DVE : (a_3*1)*a_3 stt with accum_out -> S2                (1127)
  (cnt_c comes from a host-side bincount of targets. TensorScalarPtr does
   not codegen on Pool, so Pool only gets tensor_tensor multiplies.)
"""

import numpy as np
import ml_dtypes

import concourse.bass as bass
import concourse.bacc as bacc
import concourse.tile as tile
from concourse import mybir
from concourse.bass_utils import run_bass_kernel_spmd

F32 = mybir.dt.float32
BF16 = mybir.dt.bfloat16
I32 = mybir.dt.int32
AF = mybir.ActivationFunctionType
ALU = mybir.AluOpType

B, C, H, W = 16, 4, 1024, 1024
N_CORES = 8
B_LOC = B // N_CORES
P = 128
EPS = 1e-6


def chunk_schedule(b_loc=B_LOC, h=H, w=W, n_chunk=1024):
    """Per-chunk (batch, start, size) triples covering each batch's plane.

    The last chunks taper down (960..528 wide) so the final dependency chain
    (exp->den->recip->mul->mask->square) and the DVE mask backlog shrink
    toward the end, cutting the pipeline drain tail.
    """
    free = (h * w) // P
    head = []
    tail = [960, 960, 880, 768, 528]
    chunks = []
    for b in range(b_loc):
        sizes = []
        if b == 0:
            sizes += head
        rem_tail = tail if b == b_loc - 1 else []
        body = free - sum(head if b == 0 else []) - sum(rem_tail)
        assert body % n_chunk == 0
        sizes += [n_chunk] * (body // n_chunk)
        sizes += rem_tail
        pos = 0
        for s in sizes:
            chunks.append((b, pos, s))
            pos += s
        assert pos == free
    return chunks


def build_program(b_loc=B_LOC, h=H, w=W, n_chunk=1024):
    """Build the per-core SPMD program. Returns (nc, n_chunks_total)."""
    plane = h * w
    free = plane // P
    assert free % n_chunk == 0
    chunks = chunk_schedule(b_loc, h, w, n_chunk)
    n_chunks = len(chunks)

    nc = bacc.Bacc("TRN2", target_bir_lowering=False, debug=False)

    inputs_d = nc.dram_tensor("inputs", [b_loc, C, h, w], F32, kind="ExternalInput")
    targets_d = nc.dram_tensor("targets", [b_loc, h, w], I32, kind="ExternalInput")
    out_d = nc.dram_tensor("out", [P, 6 * n_chunks], F32, kind="ExternalOutput")

    with tile.TileContext(nc) as tc:
        with (
            tc.tile_pool(name="const", bufs=1) as constp,
            tc.tile_pool(name="xs", bufs=4) as xp,
            tc.tile_pool(name="tgt", bufs=5) as tgtp,
            tc.tile_pool(name="es", bufs=4) as ep,
            tc.tile_pool(name="rs", bufs=3) as rp,
            tc.tile_pool(name="ps", bufs=3) as pp,
            tc.tile_pool(name="as_", bufs=2) as apool,
            tc.tile_pool(name="junk", bufs=2) as junkp,
            tc.tile_pool(name="stats", bufs=1) as statp,
            tc.tile_pool(name="psum", bufs=2, space="PSUM") as psump,
        ):
            # Identity weights built on-device (iota f-p == 0) so no DMA
            # ever interrupts the gapless input stream.
            ones = constp.tile([P, P], BF16)
            nc.gpsimd.memset(ones[:], 1.0)
            ident = constp.tile([P, P], BF16)
            nc.gpsimd.affine_select(
                out=ident[:], in_=ones[:], pattern=[[1, P]],
                compare_op=ALU.is_equal, fill=0.0,
                base=0, channel_multiplier=-1,
            )

            stats = statp.tile([P, 6 * n_chunks], F32, tag="stats", name="stats")

            def s1col(ci, j):
                k = ci * n_chunks + j
                return stats[:, k : k + 1]

            def s2col(ci, j):
                k = (3 + ci) * n_chunks + j
                return stats[:, k : k + 1]

            # Pipeline state carried between slots: chunk j's tiles live
            # across slots j..j+2.
            st = {}

            def stage_a(j):
                b, pos, sz = chunks[j]
                sl = slice(pos, pos + sz)
                es = []
                for c in range(C):
                    x = xp.tile([P, n_chunk], F32, tag=f"x{c}")
                    x_ap = inputs_d.ap()[b, c].rearrange("(p a) w -> p (a w)", p=P)
                    nc.sync.dma_start(x[:, :sz], x_ap[:, sl])
                    e = ep.tile([P, n_chunk], BF16, tag=f"e{c}")
                    nc.scalar.activation(e[:, :sz], x[:, :sz], AF.Exp)
                    es.append(e)
                # t is consumed two slots later (stage C); load it after the
                # x's so den is ready sooner.
                t_tile = tgtp.tile([P, n_chunk], I32, tag="t")
                tgt_ap = targets_d.ap()[b].rearrange("(p a) w -> p (a w)", p=P)
                nc.sync.dma_start(t_tile[:, :sz], tgt_ap[:, sl])
                den = psump.tile([P, n_chunk], F32, tag="den")
                for c in range(C):
                    for hh in range(0, sz, 512):
                        s2 = slice(hh, min(hh + 512, sz))
                        nc.tensor.matmul(
                            den[:, s2], ident[:], es[c][:, s2],
                            start=(c == 0), stop=(c == C - 1),
                        )
                st[j] = {"t": t_tile, "es": es, "den": den, "sz": sz}

            def stage_b(j):
                s = st[j]
                sz = s["sz"]
                r32 = rp.tile([P, n_chunk], F32, tag="r32")
                nc.vector.reciprocal_approx_fast(r32[:, :sz], s["den"][:, :sz])
                ps = []
                for ci, c in enumerate((1, 2, 3)):
                    pc = pp.tile([P, n_chunk], BF16, tag=f"pc{ci}")
                    nc.gpsimd.tensor_mul(pc[:, :sz], s["es"][c][:, :sz], r32[:, :sz])
                    ps.append(pc)
                s["ps"] = ps

            def mask_one(s, j, ci, c):
                sz = s["sz"]
                a = apool.tile([P, n_chunk], BF16, tag=f"a{ci}", name=f"a{ci}")
                nc.vector.scalar_tensor_tensor(
                    out=a[:, :sz], in0=s["t"][:, :sz], scalar=c,
                    in1=s["ps"][ci][:, :sz],
                    op0=ALU.is_equal, op1=ALU.mult,
                    accum_out=s1col(ci, j),
                )
                return a

            def stage_c1(j):
                # Class 1 only needs mul1, which finishes mid-slot: run it a
                # slot earlier than classes 2/3 to shrink the drain backlog.
                s = st[j]
                sz = s["sz"]
                a = mask_one(s, j, 0, 1)
                junk = junkp.tile([P, n_chunk], BF16, tag="junk0", name="junk0")
                nc.scalar.activation(
                    junk[:, :sz], a[:, :sz], AF.Square,
                    accum_out=s2col(0, j),
                )

            def stage_c2(j):
                s = st.pop(j)
                sz = s["sz"]
                a1 = mask_one(s, j, 1, 2)
                a2 = mask_one(s, j, 2, 3)
                junk = junkp.tile([P, n_chunk], BF16, tag="junk1", name="junk1")
                nc.scalar.activation(
                    junk[:, :sz], a1[:, :sz], AF.Square,
                    accum_out=s2col(1, j),
                )
                junk = junkp.tile([P, n_chunk], BF16, tag="junk2", name="junk2")
                nc.vector.scalar_tensor_tensor(
                    out=junk[:, :sz], in0=a2[:, :sz], scalar=1.0,
                    in1=a2[:, :sz],
                    op0=ALU.mult, op1=ALU.mult,
                    accum_out=s2col(2, j),
                )

            # Stagger the stages with logical wait timestamps so the Tile
            # scheduler bakes the decoupled pipeline order per engine (e.g.
            # DVE: recip of chunk j BEFORE masks of chunk j-1; ACT: exps
            # before squares). These only shape scheduling, not runtime.
            # The slot must exceed the scheduler-sim's own pipeline period so
            # every stage's deps complete before its floor: then the baked
            # order is exactly the floor order. Floors don't exist at
            # runtime, where semaphores pace the pipeline at the ~7.3us DMA
            # slot.
            slot_ms = 12.0e-3
            floors = [0.0]
            steps = []
            for _, _, sz in chunks:
                steps.append(slot_ms * sz / 1024.0)
                floors.append(floors[-1] + steps[-1])
            steps += [slot_ms, slot_ms]
            floors += [floors[-1] + slot_ms, floors[-1] + 2 * slot_ms]
            for j in range(n_chunks + 2):
                if j < n_chunks:
                    with tc.tile_wait_until(floors[j]):
                        stage_a(j)
                if 1 <= j <= n_chunks:
                    with tc.tile_wait_until(floors[j] + steps[j] / 3.0):
                        stage_b(j - 1)
                if j >= 2:
                    with tc.tile_wait_until(floors[j] + 2.0 * steps[j] / 3.0):
                        stage_c2(j - 2)
                if 1 <= j <= n_chunks:
                    with tc.tile_wait_until(floors[j] + 0.93 * steps[j]):
                        stage_c1(j - 1)

            nc.sync.dma_start(out_d.ap(), stats[:])

    nc.compile()
    return nc, n_chunks


_CACHED = {}


def _get_program():
    if "nc" not in _CACHED:
        _CACHED["nc"] = build_program()
    return _CACHED["nc"]


def finish_host(stats_per_core, cnt):
    """stats_per_core: list of [128, 6*n_chunks] partials; cnt: [3] counts."""
    tot = np.zeros(6, dtype=np.float64)
    for s in stats_per_core:
        n6 = s.shape[1]
        tot += s.astype(np.float64).reshape(128, 6, n6 // 6).sum(axis=(0, 2))
    s1, s2 = tot[0:3], tot[3:6]
    mean = s1 / (cnt + EPS)
    var = (s2 - 2.0 * mean * s1 + cnt * mean * mean) / (cnt + EPS)
    intra = np.where(cnt > 0, var, 0.0).sum()
    return np.float32(intra / (C - 1))


def kernel(inputs: np.ndarray, targets: np.ndarray) -> np.ndarray:
    nc, _ = _get_program()
    in_maps = [
        {
            "inputs": np.ascontiguousarray(inputs[i * B_LOC : (i + 1) * B_LOC]),
            "targets": np.ascontiguousarray(targets[i * B_LOC : (i + 1) * B_LOC]),
        }
        for i in range(N_CORES)
    ]
    res = run_bass_kernel_spmd(nc, in_maps, list(range(N_CORES)))
    stats = [res.results[i]["out"] for i in range(N_CORES)]
    cnt = np.bincount(targets.ravel(), minlength=C)[1:C].astype(np.float64)
    return finish_host(stats, cnt)
